# revision 12
# baseline (speedup 1.0000x reference)
"""Trainium2 Bass kernel for nn_BoundarySubBranch (conv1d+GN+ReLU -> deform_conv1d -> conv1d head).

Strategy:
- Data-parallel over batch: 8 samples -> 8 NeuronCores, one sample each.
- All convolutions as PE matmuls in float32r (TF32-class, full PE rate).
- Deformable sampling commuted through the channel matmul:
    dcn[:, t] = W1 @ h[:, t] + sum_j hat(pos0[t]-j) * (W0 @ h)[:, j]
                             + sum_j hat(pos2[t]-j) * (W2 @ h)[:, j]
  The hat-weight gathers become small banded matmuls with on-chip-built
  selection matrices B (exact linear-interp weights incl. zero padding).
- GroupNorm stats via ACT/DVE accumulators + tiny PE selector matmuls.
"""
import sys

sys.path.insert(0, "/opt/trn_rl_repo")

import numpy as np

import concourse.bass as bass  # noqa: F401
import concourse.tile as tile
from concourse import bacc, mybir
import concourse.bass_utils as bu

f32 = mybir.dt.float32
f32r = mybir.dt.float32r
AF = mybir.ActivationFunctionType
ALU = mybir.AluOpType

# ---------------------------------------------------------------------------
# Disable birsim inside walrus (simulation-only pass; dominates compile time).


def _fast_bir_verify(tmpdir, inp="bir.json", outp="file.neff", arch=None, *, dve_root=None):
    cmd = [
        bu.get_walrus_driver(),
        "--pass",
        ",".join(["birverifier", "runtime_memory_reservation", "lower_act",
                  "lower_dve", "lower_ap_offset", "codegen", "neff_packager"]),
        "-i", inp,
        "--neff-output-filename", outp,
        "--enable-birsim=false",
        "--mem-mode=physical",
        "--policy=0",
        "--enable-ldw-opt=false",
        "--assign-static-dmas-to-sp=false",
        "--dram-page-size=256",
        "--enable-neff-debug-info=true",
        "--jobs", "8",
        *bu.get_walrus_args(bu.get_bir_arch(tmpdir, inp) if arch is None else arch,
                            tmpdir, dve_root=dve_root),
    ]
    result = bu.run_command(cmd, cwd=tmpdir)
    if result is not None:
        from pathlib import Path
        (Path(tmpdir) / "log.txt").write_text(result.stdout)
    return f"{tmpdir}/{outp}"


bu.bir_verify_and_optimise = _fast_bir_verify

# ---------------------------------------------------------------------------
C = 512
NCORES = 8
CHUNK = 512
EPS = 1e-5
GRAD_MUL = 0.1


def _host_constants(T):
    P = 128
    ident = np.eye(P, dtype=np.float32)
    sel16 = np.zeros((P, 8), np.float32)
    for g in range(8):
        sel16[g * 16:(g + 1) * 16, g] = 1.0
    E1 = np.zeros((32, P), np.float32)
    for k in range(32):
        E1[k, 16 * (k % 8):16 * (k % 8) + 16] = 1.0
    diagm = np.zeros((32, 4), np.float32)
    for k in range(32):
        diagm[k, k // 8] = 1.0
    j = np.arange(P, dtype=np.float32)
    jbias = np.stack([-j, 128.0 - j, -128.0 - j], axis=1).astype(np.float32)
    ones_row = np.ones((1, P), np.float32)
    tmod = (np.arange(CHUNK) % 128).astype(np.float32)
    tmod2 = np.stack([tmod - 1.0, tmod + 1.0]).astype(np.float32)   # [2, CHUNK]
    sgn = np.array([[-1.0], [1.0]], np.float32)
    sel01 = np.zeros((2, 2 * P), np.float32)
    sel01[0, 0:P] = 1.0
    sel01[1, P:2 * P] = 1.0
    return dict(c_ident=ident, c_sel16=sel16, c_E1=E1, c_diagm=diagm,
                c_jbias=jbias, c_ones_row=ones_row, c_tmod2=tmod2,
                c_sgn=sgn, c_sel01=sel01)


def build(T, inv_stride):
    P = 128
    NCH = T // CHUNK
    NBLK = T // 128
    nc = bacc.Bacc("TRN2", target_bir_lowering=False, debug=False)

    feat_d = nc.dram_tensor("feat", [C, T], f32, kind="ExternalInput")
    locs_d = nc.dram_tensor("locs", [2, T], f32, kind="ExternalInput")
    conv_w_d = nc.dram_tensor("conv_w", [C, C, 3], f32, kind="ExternalInput")
    dcn_w_d = nc.dram_tensor("dcn_w", [C, C, 3], f32, kind="ExternalInput")
    out_w_d = nc.dram_tensor("out_w", [2, C, 3], f32, kind="ExternalInput")
    conv_b_d = nc.dram_tensor("conv_b", [C], f32, kind="ExternalInput")
    gn_g_d = nc.dram_tensor("gn_g", [C], f32, kind="ExternalInput")
    gn_b_d = nc.dram_tensor("gn_b", [C], f32, kind="ExternalInput")
    dcn_b_d = nc.dram_tensor("dcn_b", [C], f32, kind="ExternalInput")
    out_b_d = nc.dram_tensor("out_b", [2], f32, kind="ExternalInput")
    cst = {}
    for name, arr in _host_constants(T).items():
        cst[name] = nc.dram_tensor(name, list(arr.shape), f32, kind="ExternalInput")

    offset_d = nc.dram_tensor("offset", [2, T], f32, kind="ExternalOutput")
    offset_feat_d = nc.dram_tensor("offset_feat", [C, T], f32, kind="ExternalOutput")

    with tile.TileContext(nc) as tc:
        pc_cm = tc.tile_pool(name="const", bufs=1)
        pc = pc_cm.__enter__()
        ph_cm = tc.tile_pool(name="h", bufs=1)
        ph = ph_cm.__enter__()
        ppmm_cm = tc.tile_pool(name="ppmm", bufs=4, space="PSUM")
        ppmm = ppmm_cm.__enter__()
        pdcnT_cm = tc.tile_pool(name="dcnT", bufs=1)
        pdcnT = pdcnT_cm.__enter__()
        pcwT_cm = tc.tile_pool(name="cwT", bufs=1)
        pcwT = pcwT_cm.__enter__()
        pwraw_cm = tc.tile_pool(name="wraw", bufs=2)
        pwraw = pwraw_cm.__enter__()
        pptp_cm = tc.tile_pool(name="pptp", bufs=2, space="PSUM")
        pptp = pptp_cm.__enter__()

        # ------- constants -------
        ident = pc.tile([P, P], f32, tag="ident")
        sel16 = pc.tile([P, 8], f32, tag="sel16")
        E1 = pc.tile([32, P], f32, tag="E1")
        diagm = pc.tile([32, 4], f32, tag="diagm")
        jbias = pc.tile([P, 3], f32, tag="jbias")
        ones_row = pc.tile([1, P], f32, tag="ones_row")
        tmod2 = pc.tile([2, CHUNK], f32, tag="tmod2")
        sgn = pc.tile([2, 1], f32, tag="sgn")
        sel01 = pc.tile([2, 2 * P], f32, tag="sel01")
        for t_, d_ in [(ident, cst["c_ident"]), (sel16, cst["c_sel16"]),
                       (E1, cst["c_E1"]), (diagm, cst["c_diagm"]),
                       (jbias, cst["c_jbias"]), (ones_row, cst["c_ones_row"]),
                       (tmod2, cst["c_tmod2"]), (sgn, cst["c_sgn"]),
                       (sel01, cst["c_sel01"])]:
            nc.sync.dma_start(t_[:], d_[:])

        def load_pp(dram, tag):
            t_ = pc.tile([P, 4], f32, tag=tag, name=tag)
            nc.sync.dma_start(t_[:], dram[:].rearrange("(b p) -> p b", p=P))
            return t_
        conv_b_pp = load_pp(conv_b_d, "conv_b_pp")
        gn_g_pp = load_pp(gn_g_d, "gn_g_pp")
        gn_b_pp = load_pp(gn_b_d, "gn_b_pp")
        dcn_b_pp = load_pp(dcn_b_d, "dcn_b_pp")
        out_b_c = pc.tile([2, 1], f32, tag="out_b_c")
        nc.sync.dma_start(out_b_c[:], out_b_d[:].rearrange("(p q) -> p q", q=1))

        # ------- conv weight transpose -------
        cwT = pcwT.tile([P, 48 * P], f32r, tag="cwT")

        def transpose_weights(w_d, dst, is_conv):
            for o in range(4):
                wr = pwraw.tile([P, C, 3], f32, tag="wraw", name=f"wr{o}")
                nc.sync.dma_start(wr[:], w_d[o * P:(o + 1) * P, :, :])
                for i in range(4):
                    for k in range(3):
                        ptp = pptp.tile([P, P], f32, tag="tp", name=f"tp{o}{i}{k}")
                        nc.tensor.transpose(ptp[:], wr[:, i * P:(i + 1) * P, k], ident[:])
                        if is_conv:
                            col = (o * 12 + i * 3 + k) * P
                        else:
                            col = (k * 4 + i) * C + o * P
                        nc.scalar.copy(dst[:, col:col + P], ptp[:])

        transpose_weights(conv_w_d, cwT, True)
        dcnT = pdcnT.tile([P, 12 * C], f32r, tag="dcnT")
        owT = pdcnT.tile([P, 24], f32r, tag="owT")
        transpose_weights(dcn_w_d, dcnT, False)
        owr = pwraw.tile([2, C, 3], f32, tag="owraw")
        nc.sync.dma_start(owr[:], out_w_d[:])
        for i in range(4):
            for k in range(3):
                ptp = pptp.tile([P, 2], f32, tag="tp2", name=f"tph{i}{k}")
                nc.tensor.transpose(ptp[:], owr[:, i * P:(i + 1) * P, k], ident[0:2, 0:2])
                nc.scalar.copy(owT[:, (k * 4 + i) * 2:(k * 4 + i) * 2 + 2], ptp[:])

        pptp_cm.__exit__(None, None, None)
        pwraw_cm.__exit__(None, None, None)

        # ------- conv1 + GN stat accumulation (per-chunk x tiles) -------
        px_cm = tc.tile_pool(name="x", bufs=8)
        px = px_cm.__enter__()
        h_t = [ph.tile([P, T], f32r, tag=f"h{i}", name=f"h{i}") for i in range(4)]
        s1 = [pc.tile([P, NCH], f32, tag=f"s1_{o}", name=f"s1_{o}") for o in range(4)]
        s2 = [pc.tile([P, NCH], f32, tag=f"s2_{o}", name=f"s2_{o}") for o in range(4)]

        for c in range(NCH):
            xc = []
            lo = c * CHUNK - 1
            hi = c * CHUNK + CHUNK + 1
            for i in range(4):
                xt = px.tile([P, CHUNK + 2], f32r, tag="x", name=f"x{i}_{c}")
                dlo, dhi = 0, CHUNK + 2
                slo, shi = lo, hi
                if c == 0:
                    nc.vector.memset(xt[:, 0:1].bitcast(f32), 0.0)
                    dlo, slo = 1, 0
                if c == NCH - 1:
                    nc.vector.memset(xt[:, CHUNK + 1:CHUNK + 2].bitcast(f32), 0.0)
                    dhi, shi = CHUNK + 1, T
                nc.gpsimd.dma_start(xt[:, dlo:dhi], feat_d[i * P:(i + 1) * P, slo:shi])
                xc.append(xt)
            for o in range(4):
                pmm = ppmm.tile([P, CHUNK], f32, tag="mm", name=f"c1_{o}_{c}")
                first = True
                for i in range(4):
                    for k in range(3):
                        col = (o * 12 + i * 3 + k) * P
                        nc.tensor.matmul(pmm[:],
                                         cwT[:, col:col + P],
                                         xc[i][:, k:k + CHUNK],
                                         start=first, stop=(i == 3 and k == 2))
                        first = False
                hs = h_t[o][:, c * CHUNK:(c + 1) * CHUNK]
                nc.scalar.activation(hs, pmm[:], AF.Copy, accum_out=s1[o][:, c:c + 1])
                # squares accumulated in-place over the dead psum
                nc.vector.scalar_tensor_tensor(pmm[:], pmm[:], 1.0, hs.bitcast(f32),
                                               op0=ALU.mult, op1=ALU.mult,
                                               accum_out=s2[o][:, c:c + 1])

        px_cm.__exit__(None, None, None)
        pcwT_cm.__exit__(None, None, None)

        # ------- GN stats finalize -------
        ppmisc_cm = tc.tile_pool(name="ppmisc", bufs=2, space="PSUM")
        ppmisc = ppmisc_cm.__enter__()
        ps_stats = ppmisc.tile([1, 64], f32, tag="misc", name="ps_stats")
        for o in range(4):
            s1sum = pc.tile([P, 1], f32, tag=f"s1sum{o}", name=f"s1sum{o}")
            nc.vector.tensor_reduce(s1sum[:], s1[o][:], axis=mybir.AxisListType.X, op=ALU.add)
            s2sum = pc.tile([P, 1], f32, tag=f"s2sum{o}", name=f"s2sum{o}")
            nc.vector.tensor_reduce(s2sum[:], s2[o][:], axis=mybir.AxisListType.X, op=ALU.add)
            bcol = conv_b_pp[:, o:o + 1]
            u = pc.tile([P, 1], f32, tag=f"u{o}", name=f"u{o}")
            nc.vector.scalar_tensor_tensor(u[:], bcol, float(T), s1sum[:],
                                           op0=ALU.mult, op1=ALU.add)
            tb_ = pc.tile([P, 1], f32, tag=f"tb{o}", name=f"tb{o}")
            nc.vector.tensor_scalar_mul(tb_[:], bcol, float(T))
            q = pc.tile([P, 1], f32, tag=f"q{o}", name=f"q{o}")
            nc.vector.scalar_tensor_tensor(q[:], s1sum[:], 2.0, tb_[:],
                                           op0=ALU.mult, op1=ALU.add)
            r_ = pc.tile([P, 1], f32, tag=f"r{o}", name=f"r{o}")
            nc.vector.tensor_mul(r_[:], q[:], bcol)
            v = pc.tile([P, 1], f32, tag=f"v{o}", name=f"v{o}")
            nc.vector.tensor_add(v[:], s2sum[:], r_[:])
            nc.tensor.matmul(ps_stats[0:1, o * 8:(o + 1) * 8], u[:], sel16[:],
                             start=True, stop=True, skip_group_check=True)
            nc.tensor.matmul(ps_stats[0:1, 32 + o * 8:32 + (o + 1) * 8], v[:], sel16[:],
                             start=True, stop=True, skip_group_check=True)
        srow = pc.tile([1, 64], f32, tag="srow")
        nc.scalar.copy(srow[:], ps_stats[:])
        ps_str1 = ppmisc.tile([32, 1], f32, tag="misc", name="ps_str1")
        nc.tensor.transpose(ps_str1[:], srow[0:1, 0:32], ident[0:1, 0:1])
        scol1 = pc.tile([32, 1], f32, tag="scol1")
        nc.scalar.copy(scol1[:], ps_str1[:])
        ps_str2 = ppmisc.tile([32, 1], f32, tag="misc", name="ps_str2")
        nc.tensor.transpose(ps_str2[:], srow[0:1, 32:64], ident[0:1, 0:1])
        scol2 = pc.tile([32, 1], f32, tag="scol2")
        nc.scalar.copy(scol2[:], ps_str2[:])

        inv_n = 1.0 / (16.0 * T)
        mucol = pc.tile([32, 1], f32, tag="mucol")
        nc.vector.tensor_scalar_mul(mucol[:], scol1[:], inv_n)
        msq = pc.tile([32, 1], f32, tag="msq")
        nc.vector.tensor_mul(msq[:], mucol[:], mucol[:])
        vcol = pc.tile([32, 1], f32, tag="vcol")
        nc.vector.scalar_tensor_tensor(vcol[:], scol2[:], inv_n, msq[:],
                                       op0=ALU.mult, op1=ALU.subtract)
        vpe = pc.tile([32, 1], f32, tag="vpe")
        nc.vector.tensor_scalar_add(vpe[:], vcol[:], EPS)
        sqc = pc.tile([32, 1], f32, tag="sqc")
        nc.scalar.activation(sqc[:], vpe[:], AF.Sqrt)
        rstdc = pc.tile([32, 1], f32, tag="rstdc")
        nc.vector.reciprocal(rstdc[:], sqc[:])

        def expand(col, tag):
            rhs = pc.tile([32, 4], f32, tag=f"rhs_{tag}", name=f"rhs_{tag}")
            nc.vector.tensor_scalar(rhs[:], diagm[:], col[:], None, op0=ALU.mult)
            pex = ppmisc.tile([P, 4], f32, tag="misc", name=f"pex_{tag}")
            nc.tensor.matmul(pex[:], E1[:], rhs[:], start=True, stop=True)
            out = pc.tile([P, 4], f32, tag=f"pp_{tag}", name=f"pp_{tag}")
            nc.scalar.copy(out[:], pex[:])
            return out
        mu_pp = expand(mucol, "mu")
        rstd_pp = expand(rstdc, "rstd")

        scale_pp = pc.tile([P, 4], f32, tag="scale_pp")
        nc.vector.tensor_mul(scale_pp[:], gn_g_pp[:], rstd_pp[:])
        bias_t1 = pc.tile([P, 4], f32, tag="bias_t1")
        nc.vector.tensor_sub(bias_t1[:], conv_b_pp[:], mu_pp[:])
        bias_t2 = pc.tile([P, 4], f32, tag="bias_t2")
        nc.vector.tensor_mul(bias_t2[:], bias_t1[:], scale_pp[:])
        bias_pp = pc.tile([P, 4], f32, tag="bias_pp")
        nc.vector.tensor_add(bias_pp[:], bias_t2[:], gn_b_pp[:])

        for o in range(4):
            nc.scalar.activation(h_t[o][:], h_t[o][:].bitcast(f32), AF.Relu,
                                 bias=bias_pp[:, o:o + 1], scale=scale_pp[:, o:o + 1])

        ppmisc_cm.__exit__(None, None, None)

        # ------- DCN + head -------
        pg_cm = tc.tile_pool(name="g", bufs=7)
        pg = pg_cm.__enter__()
        pB_cm = tc.tile_pool(name="B", bufs=2)
        pB = pB_cm.__enter__()
        pof_cm = tc.tile_pool(name="of", bufs=14)
        pof = pof_cm.__enter__()
        poff_cm = tc.tile_pool(name="off", bufs=2)
        poff = poff_cm.__enter__()
        ppb_cm = tc.tile_pool(name="ppb", bufs=2, space="PSUM")
        ppb = ppb_cm.__enter__()
        pph_cm = tc.tile_pool(name="pph", bufs=2, space="PSUM")
        pph = pph_cm.__enter__()

        g0_tiles = {}
        g2_tiles = {}
        of_tiles = {}

        def make_g(k, jt, cache):
            if jt in cache or jt >= NBLK:
                return
            pmm = ppmm.tile([P, C], f32, tag="mm", name=f"g{k}_{jt}")
            for i in range(4):
                nc.tensor.matmul(pmm[:],
                                 h_t[i][:, jt * P:(jt + 1) * P],
                                 dcnT[:, (k * 4 + i) * C:(k * 4 + i + 1) * C],
                                 start=(i == 0), stop=(i == 3))
            gt = pg.tile([P, C], f32r, tag=f"g{k}", name=f"gt{k}_{jt}")
            nc.vector.tensor_copy(gt[:], pmm[:])
            cache[jt] = gt

        def head(hc):
            # fill right halos of hc's tiles (from hc+1 first col, or zero)
            for i in range(4):
                ofc = of_tiles[(i, hc)]
                if hc < NCH - 1:
                    nc.scalar.copy(ofc[:, CHUNK + 1:CHUNK + 2],
                                   of_tiles[(i, hc + 1)][:, 1:2].bitcast(f32))
                else:
                    nc.vector.memset(ofc[:, CHUNK + 1:CHUNK + 2].bitcast(f32), 0.0)
            psh = pph.tile([2, CHUNK], f32, tag="hd", name=f"hd{hc}")
            mms = []
            for k in range(3):
                for i in range(4):
                    mms.append((owT[:, (k * 4 + i) * 2:(k * 4 + i) * 2 + 2],
                                of_tiles[(i, hc)][:, k:k + CHUNK], psh[:]))
            for n_, (lhs, rhs, out) in enumerate(mms):
                nc.tensor.matmul(out, lhs, rhs, start=(n_ == 0), stop=(n_ == len(mms) - 1),
                                 skip_group_check=True)
            offt = poff.tile([2, CHUNK], f32, tag="off", name=f"off{hc}")
            nc.scalar.activation(offt[:], psh[:], AF.Identity, bias=out_b_c[:])
            nc.sync.dma_start(offset_d[:, hc * CHUNK:(hc + 1) * CHUNK], offt[:])

        for c in range(NCH):
            locs_c = pB.tile([2, CHUNK], f32, tag="locs_c", name=f"locs{c}")
            nc.sync.dma_start(locs_c[:], locs_d[:, c * CHUNK:(c + 1) * CHUNK])
            y01 = pB.tile([2, CHUNK], f32, tag="y01", name=f"y01_{c}")
            nc.vector.tensor_scalar_mul(y01[:], locs_c[:], GRAD_MUL)
            y_c = pB.tile([2, CHUNK], f32, tag="y_c", name=f"y_{c}")
            nc.vector.scalar_tensor_tensor(y_c[:], locs_c[:], 1.0 - GRAD_MUL, y01[:],
                                           op0=ALU.mult, op1=ALU.add)
            nc.vector.tensor_scalar_mul(y_c[:], y_c[:], inv_stride)
            z_c = pB.tile([2, CHUNK], f32, tag="z_c", name=f"z_{c}")
            nc.vector.tensor_scalar(z_c[:], y_c[:], sgn[:], None, op0=ALU.mult)
            pm = pB.tile([2, CHUNK], f32, tag="pm", name=f"pm_{c}")
            nc.vector.tensor_add(pm[:], z_c[:], tmod2[:])

            psb0 = ppb.tile([P, CHUNK], f32, tag="bc", name=f"psb0_{c}")
            nc.tensor.matmul(psb0[:], sel01[:, 0:P], pm[:], start=True, stop=True)
            psb2 = ppb.tile([P, CHUNK], f32, tag="bc", name=f"psb2_{c}")
            nc.tensor.matmul(psb2[:], sel01[:, P:2 * P], pm[:], start=True, stop=True)

            B0d = pB.tile([P, CHUNK], f32r, tag="B0d", name=f"B0d_{c}")
            scr = pB.tile([P, CHUNK], f32, tag="absB", name=f"sc0_{c}")
            nc.scalar.activation(scr[:], psb0[:], AF.Abs, bias=jbias[:, 0:1], scale=1.0)
            nc.scalar.activation(B0d[:], scr[:], AF.Relu, bias=1.0, scale=-1.0)
            B2d = pB.tile([P, CHUNK], f32r, tag="B2d", name=f"B2d_{c}")
            scr2 = pB.tile([P, CHUNK], f32, tag="absB", name=f"sc2_{c}")
            nc.scalar.activation(scr2[:], psb2[:], AF.Abs, bias=jbias[:, 0:1], scale=1.0)
            nc.scalar.activation(B2d[:], scr2[:], AF.Relu, bias=1.0, scale=-1.0)

            B0p = pB.tile([P, 64], f32r, tag="B0p", name=f"B0p_{c}")
            scrp = pB.tile([P, 64], f32, tag="absP", name=f"scp_{c}")
            nc.scalar.activation(scrp[:].rearrange("p (b q) -> p b q", q=16),
                                 psb0[:].rearrange("p (b q) -> p b q", q=128)[:, :, 0:16],
                                 AF.Abs, bias=jbias[:, 1:2], scale=1.0)
            nc.scalar.activation(B0p[:], scrp[:], AF.Relu, bias=1.0, scale=-1.0)
            B2n = pB.tile([P, 64], f32r, tag="B2n", name=f"B2n_{c}")
            scrn = pB.tile([P, 64], f32, tag="absP", name=f"scn_{c}")
            nc.scalar.activation(scrn[:].rearrange("p (b q) -> p b q", q=16),
                                 psb2[:].rearrange("p (b q) -> p b q", q=128)[:, :, 112:128],
                                 AF.Abs, bias=jbias[:, 2:3], scale=1.0)
            nc.scalar.activation(B2n[:], scrn[:], AF.Relu, bias=1.0, scale=-1.0)

            for b in range(4):
                make_g(0, 4 * c + b, g0_tiles)
                make_g(2, 4 * c + b, g2_tiles)
            make_g(2, 4 * c + 4, g2_tiles)

            for o in range(4):
                pdc = ppmm.tile([P, CHUNK], f32, tag="mm", name=f"dcn{c}_{o}")
                mms = []
                for i in range(4):
                    mms.append((dcnT[:, (4 + i) * C + o * P:(4 + i) * C + (o + 1) * P],
                                h_t[i][:, c * CHUNK:(c + 1) * CHUNK], pdc[:]))
                for b in range(4):
                    jt = 4 * c + b
                    gsl = slice(o * P, (o + 1) * P)
                    mms.append((g0_tiles[jt][:, gsl], B0d[:, b * P:(b + 1) * P],
                                pdc[:, b * P:(b + 1) * P]))
                    if jt > 0:
                        mms.append((g0_tiles[jt - 1][:, gsl], B0p[:, b * 16:(b + 1) * 16],
                                    pdc[:, b * P:b * P + 16]))
                    mms.append((g2_tiles[jt][:, gsl], B2d[:, b * P:(b + 1) * P],
                                pdc[:, b * P:(b + 1) * P]))
                    if jt + 1 < NBLK:
                        mms.append((g2_tiles[jt + 1][:, gsl], B2n[:, b * 16:(b + 1) * 16],
                                    pdc[:, b * P + 112:(b + 1) * P]))
                for n_, (lhs, rhs, out) in enumerate(mms):
                    nc.tensor.matmul(out, lhs, rhs, start=(n_ == 0), stop=(n_ == len(mms) - 1),
                                     skip_group_check=True)
                oft = pof.tile([P, CHUNK + 2], f32r, tag="of", name=f"of{c}_{o}")
                nc.scalar.activation(oft[:, 1:CHUNK + 1], pdc[:], AF.Relu,
                                     bias=dcn_b_pp[:, o:o + 1])
                if c == 0:
                    nc.vector.memset(oft[:, 0:1].bitcast(f32), 0.0)
                else:
                    nc.scalar.copy(oft[:, 0:1],
                                   of_tiles[(o, c - 1)][:, CHUNK:CHUNK + 1].bitcast(f32))
                of_tiles[(o, c)] = oft
                nc.sync.dma_start(offset_feat_d[o * P:(o + 1) * P, c * CHUNK:(c + 1) * CHUNK],
                                  oft[:, 1:CHUNK + 1].bitcast(f32))
            if c >= 1:
                head(c - 1)
        head(NCH - 1)

        for cm in [pph_cm, ppb_cm, poff_cm, pof_cm, pB_cm, pg_cm,
                   pdcnT_cm, ppmm_cm, ph_cm, pc_cm]:
            cm.__exit__(None, None, None)

    nc.compile()
    return nc


_CACHE = {}


def _get_nc(T, inv_stride):
    key = (T, inv_stride)
    if key not in _CACHE:
        _CACHE[key] = build(T, inv_stride)
    return _CACHE[key]


def kernel(feat, locs, conv_w, conv_b, gn_g, gn_b, dcn_w, dcn_b, out_w, out_b, stride):
    from concourse.bass_utils import run_bass_kernel_spmd
    feat = np.ascontiguousarray(np.asarray(feat, dtype=np.float32))
    locs = np.ascontiguousarray(np.asarray(locs, dtype=np.float32))
    N, C_, T = feat.shape
    assert N == NCORES and C_ == C
    inv_stride = float(1.0 / float(np.asarray(stride)))
    nc = _get_nc(T, inv_stride)

    shared = {
        "conv_w": np.ascontiguousarray(np.asarray(conv_w, np.float32)),
        "dcn_w": np.ascontiguousarray(np.asarray(dcn_w, np.float32)),
        "out_w": np.ascontiguousarray(np.asarray(out_w, np.float32)),
        "conv_b": np.ascontiguousarray(np.asarray(conv_b, np.float32)),
        "gn_g": np.ascontiguousarray(np.asarray(gn_g, np.float32)),
        "gn_b": np.ascontiguousarray(np.asarray(gn_b, np.float32)),
        "dcn_b": np.ascontiguousarray(np.asarray(dcn_b, np.float32)),
        "out_b": np.ascontiguousarray(np.asarray(out_b, np.float32)),
    }
    shared.update(_host_constants(T))

    in_maps = []
    for i in range(NCORES):
        m = dict(shared)
        m["feat"] = np.ascontiguousarray(feat[i])
        m["locs"] = np.ascontiguousarray(locs[i])
        in_maps.append(m)

    res = run_bass_kernel_spmd(nc, in_maps, core_ids=list(range(NCORES)))
    offset = np.stack([res.results[i]["offset"] for i in range(NCORES)])
    offset_feat = np.stack([res.results[i]["offset_feat"] for i in range(NCORES)])
    return (offset, offset_feat)


# revision 13
# speedup vs baseline: 1.0216x; 1.0216x over previous
"""Trainium2 Bass kernel for nn_BoundarySubBranch (conv1d+GN+ReLU -> deform_conv1d -> conv1d head).

Strategy:
- Data-parallel over batch: 8 samples -> 8 NeuronCores, one sample each.
- All convolutions as PE matmuls in float32r (TF32-class, full PE rate).
- Deformable sampling commuted through the channel matmul:
    dcn[:, t] = W1 @ h[:, t] + sum_j hat(pos0[t]-j) * (W0 @ h)[:, j]
                             + sum_j hat(pos2[t]-j) * (W2 @ h)[:, j]
  The hat-weight gathers become small banded matmuls with on-chip-built
  selection matrices B (exact linear-interp weights incl. zero padding).
- GroupNorm stats via ACT/DVE accumulators + tiny PE selector matmuls.
"""
import sys

sys.path.insert(0, "/opt/trn_rl_repo")

import numpy as np

import concourse.bass as bass  # noqa: F401
import concourse.tile as tile
from concourse import bacc, mybir
import concourse.bass_utils as bu

f32 = mybir.dt.float32
f32r = mybir.dt.float32r
AF = mybir.ActivationFunctionType
ALU = mybir.AluOpType

# ---------------------------------------------------------------------------
# Disable birsim inside walrus (simulation-only pass; dominates compile time).


def _fast_bir_verify(tmpdir, inp="bir.json", outp="file.neff", arch=None, *, dve_root=None):
    cmd = [
        bu.get_walrus_driver(),
        "--pass",
        ",".join(["birverifier", "runtime_memory_reservation", "lower_act",
                  "lower_dve", "lower_ap_offset", "codegen", "neff_packager"]),
        "-i", inp,
        "--neff-output-filename", outp,
        "--enable-birsim=false",
        "--mem-mode=physical",
        "--policy=0",
        "--enable-ldw-opt=false",
        "--assign-static-dmas-to-sp=false",
        "--dram-page-size=256",
        "--enable-neff-debug-info=true",
        "--jobs", "8",
        *bu.get_walrus_args(bu.get_bir_arch(tmpdir, inp) if arch is None else arch,
                            tmpdir, dve_root=dve_root),
    ]
    result = bu.run_command(cmd, cwd=tmpdir)
    if result is not None:
        from pathlib import Path
        (Path(tmpdir) / "log.txt").write_text(result.stdout)
    return f"{tmpdir}/{outp}"


bu.bir_verify_and_optimise = _fast_bir_verify

# ---------------------------------------------------------------------------
C = 512
NCORES = 8
CHUNK = 512
EPS = 1e-5
GRAD_MUL = 0.1


def _host_constants(T):
    P = 128
    ident = np.eye(P, dtype=np.float32)
    sel16 = np.zeros((P, 8), np.float32)
    for g in range(8):
        sel16[g * 16:(g + 1) * 16, g] = 1.0
    E1 = np.zeros((32, P), np.float32)
    for k in range(32):
        E1[k, 16 * (k % 8):16 * (k % 8) + 16] = 1.0
    diagm = np.zeros((32, 4), np.float32)
    for k in range(32):
        diagm[k, k // 8] = 1.0
    j = np.arange(P, dtype=np.float32)
    jbias = np.stack([-j, 128.0 - j, -128.0 - j], axis=1).astype(np.float32)
    ones_row = np.ones((1, P), np.float32)
    tmod = (np.arange(CHUNK) % 128).astype(np.float32)
    tmod2 = np.stack([tmod - 1.0, tmod + 1.0]).astype(np.float32)   # [2, CHUNK]
    sgn = np.array([[-1.0], [1.0]], np.float32)
    sel01 = np.zeros((2, 2 * P), np.float32)
    sel01[0, 0:P] = 1.0
    sel01[1, P:2 * P] = 1.0
    return dict(c_ident=ident, c_sel16=sel16, c_E1=E1, c_diagm=diagm,
                c_jbias=jbias, c_ones_row=ones_row, c_tmod2=tmod2,
                c_sgn=sgn, c_sel01=sel01)


def build(T, inv_stride):
    P = 128
    NCH = T // CHUNK
    NBLK = T // 128
    nc = bacc.Bacc("TRN2", target_bir_lowering=False, debug=False)

    feat_d = nc.dram_tensor("feat", [C, T], f32, kind="ExternalInput")
    locs_d = nc.dram_tensor("locs", [2, T], f32, kind="ExternalInput")
    conv_w_d = nc.dram_tensor("conv_w", [C, C, 3], f32, kind="ExternalInput")
    dcn_w_d = nc.dram_tensor("dcn_w", [C, C, 3], f32, kind="ExternalInput")
    out_w_d = nc.dram_tensor("out_w", [2, C, 3], f32, kind="ExternalInput")
    conv_b_d = nc.dram_tensor("conv_b", [C], f32, kind="ExternalInput")
    gn_g_d = nc.dram_tensor("gn_g", [C], f32, kind="ExternalInput")
    gn_b_d = nc.dram_tensor("gn_b", [C], f32, kind="ExternalInput")
    dcn_b_d = nc.dram_tensor("dcn_b", [C], f32, kind="ExternalInput")
    out_b_d = nc.dram_tensor("out_b", [2], f32, kind="ExternalInput")
    cst = {}
    for name, arr in _host_constants(T).items():
        cst[name] = nc.dram_tensor(name, list(arr.shape), f32, kind="ExternalInput")

    offset_d = nc.dram_tensor("offset", [2, T], f32, kind="ExternalOutput")
    offset_feat_d = nc.dram_tensor("offset_feat", [C, T], f32, kind="ExternalOutput")

    with tile.TileContext(nc) as tc:
        pc_cm = tc.tile_pool(name="const", bufs=1)
        pc = pc_cm.__enter__()
        ph_cm = tc.tile_pool(name="h", bufs=1)
        ph = ph_cm.__enter__()
        ppmm_cm = tc.tile_pool(name="ppmm", bufs=4, space="PSUM")
        ppmm = ppmm_cm.__enter__()
        pdcnT_cm = tc.tile_pool(name="dcnT", bufs=1)
        pdcnT = pdcnT_cm.__enter__()
        pcwT_cm = tc.tile_pool(name="cwT", bufs=1)
        pcwT = pcwT_cm.__enter__()
        pwraw_cm = tc.tile_pool(name="wraw", bufs=2)
        pwraw = pwraw_cm.__enter__()
        pptp_cm = tc.tile_pool(name="pptp", bufs=2, space="PSUM")
        pptp = pptp_cm.__enter__()

        # ------- constants -------
        ident = pc.tile([P, P], f32, tag="ident")
        sel16 = pc.tile([P, 8], f32, tag="sel16")
        E1 = pc.tile([32, P], f32, tag="E1")
        diagm = pc.tile([32, 4], f32, tag="diagm")
        jbias = pc.tile([P, 3], f32, tag="jbias")
        ones_row = pc.tile([1, P], f32, tag="ones_row")
        tmod2 = pc.tile([2, CHUNK], f32, tag="tmod2")
        sgn = pc.tile([2, 1], f32, tag="sgn")
        sel01 = pc.tile([2, 2 * P], f32, tag="sel01")
        for t_, d_ in [(ident, cst["c_ident"]), (sel16, cst["c_sel16"]),
                       (E1, cst["c_E1"]), (diagm, cst["c_diagm"]),
                       (jbias, cst["c_jbias"]), (ones_row, cst["c_ones_row"]),
                       (tmod2, cst["c_tmod2"]), (sgn, cst["c_sgn"]),
                       (sel01, cst["c_sel01"])]:
            nc.sync.dma_start(t_[:], d_[:])

        def load_pp(dram, tag):
            t_ = pc.tile([P, 4], f32, tag=tag, name=tag)
            nc.sync.dma_start(t_[:], dram[:].rearrange("(b p) -> p b", p=P))
            return t_
        conv_b_pp = load_pp(conv_b_d, "conv_b_pp")
        gn_g_pp = load_pp(gn_g_d, "gn_g_pp")
        gn_b_pp = load_pp(gn_b_d, "gn_b_pp")
        dcn_b_pp = load_pp(dcn_b_d, "dcn_b_pp")
        out_b_c = pc.tile([2, 1], f32, tag="out_b_c")
        nc.sync.dma_start(out_b_c[:], out_b_d[:].rearrange("(p q) -> p q", q=1))

        # ------- conv weight transpose -------
        cwT = pcwT.tile([P, 48 * P], f32r, tag="cwT")

        def transpose_weights(w_d, dst, is_conv):
            for o in range(4):
                wr = pwraw.tile([P, C, 3], f32, tag="wraw", name=f"wr{o}")
                nc.sync.dma_start(wr[:], w_d[o * P:(o + 1) * P, :, :])
                for i in range(4):
                    for k in range(3):
                        ptp = pptp.tile([P, P], f32, tag="tp", name=f"tp{o}{i}{k}")
                        nc.tensor.transpose(ptp[:], wr[:, i * P:(i + 1) * P, k], ident[:])
                        if is_conv:
                            col = (o * 12 + i * 3 + k) * P
                        else:
                            col = (k * 4 + i) * C + o * P
                        nc.scalar.copy(dst[:, col:col + P], ptp[:])

        transpose_weights(conv_w_d, cwT, True)
        dcnT = pdcnT.tile([P, 12 * C], f32r, tag="dcnT")
        owT = pdcnT.tile([P, 24], f32r, tag="owT")
        transpose_weights(dcn_w_d, dcnT, False)
        owr = pwraw.tile([2, C, 3], f32, tag="owraw")
        nc.sync.dma_start(owr[:], out_w_d[:])
        for i in range(4):
            for k in range(3):
                ptp = pptp.tile([P, 2], f32, tag="tp2", name=f"tph{i}{k}")
                nc.tensor.transpose(ptp[:], owr[:, i * P:(i + 1) * P, k], ident[0:2, 0:2])
                nc.scalar.copy(owT[:, (k * 4 + i) * 2:(k * 4 + i) * 2 + 2], ptp[:])

        pptp_cm.__exit__(None, None, None)
        pwraw_cm.__exit__(None, None, None)

        # ------- conv1 + GN stat accumulation (per-chunk x tiles) -------
        px_cm = tc.tile_pool(name="x", bufs=8)
        px = px_cm.__enter__()
        h_t = [ph.tile([P, T], f32r, tag=f"h{i}", name=f"h{i}") for i in range(4)]
        s1 = [pc.tile([P, NCH], f32, tag=f"s1_{o}", name=f"s1_{o}") for o in range(4)]
        s2 = [pc.tile([P, NCH], f32, tag=f"s2_{o}", name=f"s2_{o}") for o in range(4)]

        for c in range(NCH):
            xc = []
            lo = c * CHUNK - 1
            hi = c * CHUNK + CHUNK + 1
            for i in range(4):
                xt = px.tile([P, CHUNK + 2], f32r, tag="x", name=f"x{i}_{c}")
                dlo, dhi = 0, CHUNK + 2
                slo, shi = lo, hi
                if c == 0:
                    nc.vector.memset(xt[:, 0:1].bitcast(f32), 0.0)
                    dlo, slo = 1, 0
                if c == NCH - 1:
                    nc.vector.memset(xt[:, CHUNK + 1:CHUNK + 2].bitcast(f32), 0.0)
                    dhi, shi = CHUNK + 1, T
                nc.gpsimd.dma_start(xt[:, dlo:dhi], feat_d[i * P:(i + 1) * P, slo:shi])
                xc.append(xt)
            for o in range(4):
                pmm = ppmm.tile([P, CHUNK], f32, tag="mm", name=f"c1_{o}_{c}")
                first = True
                for i in range(4):
                    for k in range(3):
                        col = (o * 12 + i * 3 + k) * P
                        nc.tensor.matmul(pmm[:],
                                         cwT[:, col:col + P],
                                         xc[i][:, k:k + CHUNK],
                                         start=first, stop=(i == 3 and k == 2))
                        first = False
                hs = h_t[o][:, c * CHUNK:(c + 1) * CHUNK]
                nc.scalar.activation(hs, pmm[:], AF.Copy, accum_out=s1[o][:, c:c + 1])
                # squares accumulated in-place over the dead psum
                nc.vector.scalar_tensor_tensor(pmm[:], pmm[:], 1.0, hs.bitcast(f32),
                                               op0=ALU.mult, op1=ALU.mult,
                                               accum_out=s2[o][:, c:c + 1])

        px_cm.__exit__(None, None, None)
        pcwT_cm.__exit__(None, None, None)

        # ------- GN stats finalize -------
        ppmisc_cm = tc.tile_pool(name="ppmisc", bufs=2, space="PSUM")
        ppmisc = ppmisc_cm.__enter__()
        ps_stats = ppmisc.tile([1, 64], f32, tag="misc", name="ps_stats")
        for o in range(4):
            s1sum = pc.tile([P, 1], f32, tag=f"s1sum{o}", name=f"s1sum{o}")
            nc.vector.tensor_reduce(s1sum[:], s1[o][:], axis=mybir.AxisListType.X, op=ALU.add)
            s2sum = pc.tile([P, 1], f32, tag=f"s2sum{o}", name=f"s2sum{o}")
            nc.vector.tensor_reduce(s2sum[:], s2[o][:], axis=mybir.AxisListType.X, op=ALU.add)
            bcol = conv_b_pp[:, o:o + 1]
            u = pc.tile([P, 1], f32, tag=f"u{o}", name=f"u{o}")
            nc.vector.scalar_tensor_tensor(u[:], bcol, float(T), s1sum[:],
                                           op0=ALU.mult, op1=ALU.add)
            tb_ = pc.tile([P, 1], f32, tag=f"tb{o}", name=f"tb{o}")
            nc.vector.tensor_scalar_mul(tb_[:], bcol, float(T))
            q = pc.tile([P, 1], f32, tag=f"q{o}", name=f"q{o}")
            nc.vector.scalar_tensor_tensor(q[:], s1sum[:], 2.0, tb_[:],
                                           op0=ALU.mult, op1=ALU.add)
            r_ = pc.tile([P, 1], f32, tag=f"r{o}", name=f"r{o}")
            nc.vector.tensor_mul(r_[:], q[:], bcol)
            v = pc.tile([P, 1], f32, tag=f"v{o}", name=f"v{o}")
            nc.vector.tensor_add(v[:], s2sum[:], r_[:])
            nc.tensor.matmul(ps_stats[0:1, o * 8:(o + 1) * 8], u[:], sel16[:],
                             start=True, stop=True, skip_group_check=True)
            nc.tensor.matmul(ps_stats[0:1, 32 + o * 8:32 + (o + 1) * 8], v[:], sel16[:],
                             start=True, stop=True, skip_group_check=True)
        srow = pc.tile([1, 64], f32, tag="srow")
        nc.scalar.copy(srow[:], ps_stats[:])
        ps_str1 = ppmisc.tile([32, 1], f32, tag="misc", name="ps_str1")
        nc.tensor.transpose(ps_str1[:], srow[0:1, 0:32], ident[0:1, 0:1])
        scol1 = pc.tile([32, 1], f32, tag="scol1")
        nc.scalar.copy(scol1[:], ps_str1[:])
        ps_str2 = ppmisc.tile([32, 1], f32, tag="misc", name="ps_str2")
        nc.tensor.transpose(ps_str2[:], srow[0:1, 32:64], ident[0:1, 0:1])
        scol2 = pc.tile([32, 1], f32, tag="scol2")
        nc.scalar.copy(scol2[:], ps_str2[:])

        inv_n = 1.0 / (16.0 * T)
        mucol = pc.tile([32, 1], f32, tag="mucol")
        nc.vector.tensor_scalar_mul(mucol[:], scol1[:], inv_n)
        msq = pc.tile([32, 1], f32, tag="msq")
        nc.vector.tensor_mul(msq[:], mucol[:], mucol[:])
        vcol = pc.tile([32, 1], f32, tag="vcol")
        nc.vector.scalar_tensor_tensor(vcol[:], scol2[:], inv_n, msq[:],
                                       op0=ALU.mult, op1=ALU.subtract)
        vpe = pc.tile([32, 1], f32, tag="vpe")
        nc.vector.tensor_scalar_add(vpe[:], vcol[:], EPS)
        sqc = pc.tile([32, 1], f32, tag="sqc")
        nc.scalar.activation(sqc[:], vpe[:], AF.Sqrt)
        rstdc = pc.tile([32, 1], f32, tag="rstdc")
        nc.vector.reciprocal(rstdc[:], sqc[:])

        def expand(col, tag):
            rhs = pc.tile([32, 4], f32, tag=f"rhs_{tag}", name=f"rhs_{tag}")
            nc.vector.tensor_scalar(rhs[:], diagm[:], col[:], None, op0=ALU.mult)
            pex = ppmisc.tile([P, 4], f32, tag="misc", name=f"pex_{tag}")
            nc.tensor.matmul(pex[:], E1[:], rhs[:], start=True, stop=True)
            out = pc.tile([P, 4], f32, tag=f"pp_{tag}", name=f"pp_{tag}")
            nc.scalar.copy(out[:], pex[:])
            return out
        mu_pp = expand(mucol, "mu")
        rstd_pp = expand(rstdc, "rstd")

        scale_pp = pc.tile([P, 4], f32, tag="scale_pp")
        nc.vector.tensor_mul(scale_pp[:], gn_g_pp[:], rstd_pp[:])
        bias_t1 = pc.tile([P, 4], f32, tag="bias_t1")
        nc.vector.tensor_sub(bias_t1[:], conv_b_pp[:], mu_pp[:])
        bias_t2 = pc.tile([P, 4], f32, tag="bias_t2")
        nc.vector.tensor_mul(bias_t2[:], bias_t1[:], scale_pp[:])
        bias_pp = pc.tile([P, 4], f32, tag="bias_pp")
        nc.vector.tensor_add(bias_pp[:], bias_t2[:], gn_b_pp[:])

        for o in range(4):
            nc.scalar.activation(h_t[o][:], h_t[o][:].bitcast(f32), AF.Relu,
                                 bias=bias_pp[:, o:o + 1], scale=scale_pp[:, o:o + 1])

        ppmisc_cm.__exit__(None, None, None)

        # ------- DCN + head -------
        pg_cm = tc.tile_pool(name="g", bufs=7)
        pg = pg_cm.__enter__()
        pB_cm = tc.tile_pool(name="B", bufs=2)
        pB = pB_cm.__enter__()
        pof_cm = tc.tile_pool(name="of", bufs=14)
        pof = pof_cm.__enter__()
        poff_cm = tc.tile_pool(name="off", bufs=2)
        poff = poff_cm.__enter__()
        ppb_cm = tc.tile_pool(name="ppb", bufs=2, space="PSUM")
        ppb = ppb_cm.__enter__()
        pph_cm = tc.tile_pool(name="pph", bufs=2, space="PSUM")
        pph = pph_cm.__enter__()

        g0_tiles = {}
        g2_tiles = {}
        of_tiles = {}

        def make_g(k, jt, cache):
            if jt in cache or jt >= NBLK:
                return
            pmm = ppmm.tile([P, C], f32, tag="mm", name=f"g{k}_{jt}")
            for i in range(4):
                nc.tensor.matmul(pmm[:],
                                 h_t[i][:, jt * P:(jt + 1) * P],
                                 dcnT[:, (k * 4 + i) * C:(k * 4 + i + 1) * C],
                                 start=(i == 0), stop=(i == 3))
            gt = pg.tile([P, C], f32r, tag=f"g{k}", name=f"gt{k}_{jt}")
            nc.vector.tensor_copy(gt[:], pmm[:])
            cache[jt] = gt

        def head(hc):
            # fill right halos of hc's tiles (from hc+1 first col, or zero)
            for i in range(4):
                ofc = of_tiles[(i, hc)]
                if hc < NCH - 1:
                    nc.scalar.copy(ofc[:, CHUNK + 1:CHUNK + 2],
                                   of_tiles[(i, hc + 1)][:, 1:2].bitcast(f32))
                else:
                    nc.vector.memset(ofc[:, CHUNK + 1:CHUNK + 2].bitcast(f32), 0.0)
            psh = pph.tile([2, CHUNK], f32, tag="hd", name=f"hd{hc}")
            mms = []
            for k in range(3):
                for i in range(4):
                    mms.append((owT[:, (k * 4 + i) * 2:(k * 4 + i) * 2 + 2],
                                of_tiles[(i, hc)][:, k:k + CHUNK], psh[:]))
            for n_, (lhs, rhs, out) in enumerate(mms):
                nc.tensor.matmul(out, lhs, rhs, start=(n_ == 0), stop=(n_ == len(mms) - 1),
                                 skip_group_check=True)
            offt = poff.tile([2, CHUNK], f32, tag="off", name=f"off{hc}")
            nc.scalar.activation(offt[:], psh[:], AF.Identity, bias=out_b_c[:])
            nc.sync.dma_start(offset_d[:, hc * CHUNK:(hc + 1) * CHUNK], offt[:])

        for c in range(NCH):
            locs_c = pB.tile([2, CHUNK], f32, tag="locs_c", name=f"locs{c}")
            nc.sync.dma_start(locs_c[:], locs_d[:, c * CHUNK:(c + 1) * CHUNK])
            y01 = pB.tile([2, CHUNK], f32, tag="y01", name=f"y01_{c}")
            nc.vector.tensor_scalar_mul(y01[:], locs_c[:], GRAD_MUL)
            y_c = pB.tile([2, CHUNK], f32, tag="y_c", name=f"y_{c}")
            nc.vector.scalar_tensor_tensor(y_c[:], locs_c[:], 1.0 - GRAD_MUL, y01[:],
                                           op0=ALU.mult, op1=ALU.add)
            nc.vector.tensor_scalar_mul(y_c[:], y_c[:], inv_stride)
            z_c = pB.tile([2, CHUNK], f32, tag="z_c", name=f"z_{c}")
            nc.vector.tensor_scalar(z_c[:], y_c[:], sgn[:], None, op0=ALU.mult)
            pm = pB.tile([2, CHUNK], f32, tag="pm", name=f"pm_{c}")
            nc.vector.tensor_add(pm[:], z_c[:], tmod2[:])

            psb0 = ppb.tile([P, CHUNK], f32, tag="bc", name=f"psb0_{c}")
            nc.tensor.matmul(psb0[:], sel01[:, 0:P], pm[:], start=True, stop=True)
            psb2 = ppb.tile([P, CHUNK], f32, tag="bc", name=f"psb2_{c}")
            nc.tensor.matmul(psb2[:], sel01[:, P:2 * P], pm[:], start=True, stop=True)

            B0d = pB.tile([P, CHUNK], f32r, tag="B0d", name=f"B0d_{c}")
            scr = pB.tile([P, CHUNK], f32, tag="absB", name=f"sc0_{c}")
            nc.scalar.activation(scr[:], psb0[:], AF.Abs, bias=jbias[:, 0:1], scale=1.0)
            nc.scalar.activation(B0d[:], scr[:], AF.Relu, bias=1.0, scale=-1.0)
            B2d = pB.tile([P, CHUNK], f32r, tag="B2d", name=f"B2d_{c}")
            scr2 = pB.tile([P, CHUNK], f32, tag="absB", name=f"sc2_{c}")
            nc.scalar.activation(scr2[:], psb2[:], AF.Abs, bias=jbias[:, 0:1], scale=1.0)
            nc.scalar.activation(B2d[:], scr2[:], AF.Relu, bias=1.0, scale=-1.0)

            B0p = pB.tile([P, 64], f32r, tag="B0p", name=f"B0p_{c}")
            scrp = pB.tile([P, 64], f32, tag="absP", name=f"scp_{c}")
            nc.scalar.activation(scrp[:].rearrange("p (b q) -> p b q", q=16),
                                 psb0[:].rearrange("p (b q) -> p b q", q=128)[:, :, 0:16],
                                 AF.Abs, bias=jbias[:, 1:2], scale=1.0)
            nc.scalar.activation(B0p[:], scrp[:], AF.Relu, bias=1.0, scale=-1.0)
            B2n = pB.tile([P, 64], f32r, tag="B2n", name=f"B2n_{c}")
            scrn = pB.tile([P, 64], f32, tag="absP", name=f"scn_{c}")
            nc.scalar.activation(scrn[:].rearrange("p (b q) -> p b q", q=16),
                                 psb2[:].rearrange("p (b q) -> p b q", q=128)[:, :, 112:128],
                                 AF.Abs, bias=jbias[:, 2:3], scale=1.0)
            nc.scalar.activation(B2n[:], scrn[:], AF.Relu, bias=1.0, scale=-1.0)

            for b in range(4):
                make_g(0, 4 * c + b, g0_tiles)
                make_g(2, 4 * c + b, g2_tiles)
            make_g(2, 4 * c + 4, g2_tiles)

            for o in range(4):
                pdc = ppmm.tile([P, CHUNK], f32, tag="mm", name=f"dcn{c}_{o}")
                mms = []
                for i in range(4):
                    mms.append((dcnT[:, (4 + i) * C + o * P:(4 + i) * C + (o + 1) * P],
                                h_t[i][:, c * CHUNK:(c + 1) * CHUNK], pdc[:]))
                for b in range(4):
                    jt = 4 * c + b
                    gsl = slice(o * P, (o + 1) * P)
                    mms.append((g0_tiles[jt][:, gsl], B0d[:, b * P:(b + 1) * P],
                                pdc[:, b * P:(b + 1) * P]))
                    if jt > 0:
                        mms.append((g0_tiles[jt - 1][:, gsl], B0p[:, b * 16:(b + 1) * 16],
                                    pdc[:, b * P:b * P + 16]))
                    mms.append((g2_tiles[jt][:, gsl], B2d[:, b * P:(b + 1) * P],
                                pdc[:, b * P:(b + 1) * P]))
                    if jt + 1 < NBLK:
                        mms.append((g2_tiles[jt + 1][:, gsl], B2n[:, b * 16:(b + 1) * 16],
                                    pdc[:, b * P + 112:(b + 1) * P]))
                for n_, (lhs, rhs, out) in enumerate(mms):
                    nc.tensor.matmul(out, lhs, rhs, start=(n_ == 0), stop=(n_ == len(mms) - 1),
                                     skip_group_check=True)
                oft = pof.tile([P, CHUNK + 2], f32r, tag="of", name=f"of{c}_{o}")
                nc.scalar.activation(oft[:, 1:CHUNK + 1], pdc[:], AF.Relu,
                                     bias=dcn_b_pp[:, o:o + 1])
                if c == 0:
                    nc.vector.memset(oft[:, 0:1].bitcast(f32), 0.0)
                else:
                    nc.scalar.copy(oft[:, 0:1],
                                   of_tiles[(o, c - 1)][:, CHUNK:CHUNK + 1].bitcast(f32))
                of_tiles[(o, c)] = oft
                nc.sync.dma_start(offset_feat_d[o * P:(o + 1) * P, c * CHUNK:(c + 1) * CHUNK],
                                  oft[:, 1:CHUNK + 1].bitcast(f32))
            if c >= 1:
                head(c - 1)
        head(NCH - 1)

        for cm in [pph_cm, ppb_cm, poff_cm, pof_cm, pB_cm, pg_cm,
                   pdcnT_cm, ppmm_cm, ph_cm, pc_cm]:
            cm.__exit__(None, None, None)

    nc.compile()
    return nc


_CACHE = {}


def _get_nc(T, inv_stride):
    key = (T, inv_stride)
    if key not in _CACHE:
        _CACHE[key] = build(T, inv_stride)
    return _CACHE[key]


def _make_runner(nc, n_cores):
    """Cached shard_map runner (compiles the NEFF once, reuses executable)."""
    import jax
    import numpy as _np
    from jax.sharding import Mesh, PartitionSpec
    from jax.experimental.shard_map import shard_map
    from concourse import bass2jax as b2j
    from concourse import mybir as _mybir

    b2j.install_neuronx_cc_hook()
    partition_name = nc.partition_id_tensor.name if nc.partition_id_tensor else None
    in_names, out_names, out_avals, zero_shapes = [], [], [], []
    for alloc in nc.m.functions[0].allocations:
        if not isinstance(alloc, _mybir.MemoryLocationSet):
            continue
        name = alloc.memorylocations[0].name
        if alloc.kind == "ExternalInput":
            if name != partition_name:
                in_names.append(name)
        elif alloc.kind == "ExternalOutput":
            out_names.append(name)
            shape = tuple(alloc.tensor_shape)
            dtype = _mybir.dt.np(alloc.dtype)
            out_avals.append(jax.core.ShapedArray(shape, dtype))
            zero_shapes.append((shape, dtype))
    n_params = len(in_names)
    n_outs = len(out_avals)
    all_in_names = list(in_names) + list(out_names)
    if partition_name is not None:
        all_in_names.append(partition_name)
    donate = tuple(range(n_params, n_params + n_outs))

    def _body(*args):
        operands = list(args)
        if partition_name is not None:
            operands.append(b2j.partition_id_tensor())
        outs = b2j._bass_exec_p.bind(
            *operands,
            out_avals=tuple(out_avals),
            in_names=tuple(all_in_names),
            out_names=tuple(out_names),
            lowering_input_output_aliases=(),
            sim_require_finite=True,
            sim_require_nnan=True,
            nc=nc,
        )
        return tuple(outs)

    devices = jax.devices()[:n_cores]
    mesh = Mesh(_np.asarray(devices), ("core",))
    in_specs = (PartitionSpec("core"),) * (n_params + n_outs)
    out_specs = (PartitionSpec("core"),) * n_outs
    sharded = jax.jit(
        shard_map(_body, mesh=mesh, in_specs=in_specs, out_specs=out_specs,
                  check_rep=False),
        donate_argnums=donate, keep_unused=True)

    def run(in_maps):
        per_core = [[_np.asarray(m[name]) for name in in_names] for m in in_maps]
        concat_in = [_np.concatenate([per_core[c][i] for c in range(n_cores)], axis=0)
                     for i in range(n_params)]
        concat_zeros = [_np.zeros((n_cores * s[0], *s[1:]), d) for s, d in zero_shapes]
        out_arrs = sharded(*concat_in, *concat_zeros)
        out_arrs = [_np.asarray(a) for a in out_arrs]
        return [
            {name: out_arrs[i].reshape(n_cores, *out_avals[i].shape)[c]
             for i, name in enumerate(out_names)}
            for c in range(n_cores)
        ]

    return run


_RUNNERS = {}


def _get_runner(T, inv_stride):
    key = (T, inv_stride)
    if key not in _RUNNERS:
        nc = _get_nc(T, inv_stride)
        _RUNNERS[key] = _make_runner(nc, NCORES)
    return _RUNNERS[key]


def kernel(feat, locs, conv_w, conv_b, gn_g, gn_b, dcn_w, dcn_b, out_w, out_b, stride):
    from concourse.bass_utils import run_bass_kernel_spmd  # noqa: F401
    feat = np.ascontiguousarray(np.asarray(feat, dtype=np.float32))
    locs = np.ascontiguousarray(np.asarray(locs, dtype=np.float32))
    N, C_, T = feat.shape
    assert N == NCORES and C_ == C
    inv_stride = float(1.0 / float(np.asarray(stride)))

    shared = {
        "conv_w": np.ascontiguousarray(np.asarray(conv_w, np.float32)),
        "dcn_w": np.ascontiguousarray(np.asarray(dcn_w, np.float32)),
        "out_w": np.ascontiguousarray(np.asarray(out_w, np.float32)),
        "conv_b": np.ascontiguousarray(np.asarray(conv_b, np.float32)),
        "gn_g": np.ascontiguousarray(np.asarray(gn_g, np.float32)),
        "gn_b": np.ascontiguousarray(np.asarray(gn_b, np.float32)),
        "dcn_b": np.ascontiguousarray(np.asarray(dcn_b, np.float32)),
        "out_b": np.ascontiguousarray(np.asarray(out_b, np.float32)),
    }
    shared.update(_host_constants(T))

    in_maps = []
    for i in range(NCORES):
        m = dict(shared)
        m["feat"] = np.ascontiguousarray(feat[i])
        m["locs"] = np.ascontiguousarray(locs[i])
        in_maps.append(m)

    run = _get_runner(T, inv_stride)
    results = run(in_maps)
    offset = np.stack([results[i]["offset"] for i in range(NCORES)])
    offset_feat = np.stack([results[i]["offset_feat"] for i in range(NCORES)])
    return (offset, offset_feat)
